# revision 6
# baseline (speedup 1.0000x reference)
"""HMM scaled-forward (alpha scaling) kernel for Trainium2, 8 NeuronCores.

Math: alpha_t = normalize((alpha_{t-1} @ A) * b[:, x_t]).
The map v -> normalize((v @ A) * e) is a Hilbert-metric contraction (A is a
dense positive stochastic matrix; diagonal emission scaling is an isometry),
so the T=1M sequential scan is split into 8*320 independent chains, each
seeded by a 32-step host-side warmup (empirically converges to fp32 machine
precision in ~16 steps). Per-step normalization is dropped on device
(prescaled emissions keep the unnormalized state within e^{+-10} over a
391-step chain); rows are normalized on the host at the end.

Device per step: S_t = (S_{t-1}^T @ blockdiag(A,A)) * E_t   (PE + DVE),
history is transposed per chain-pair on the PE into output-row layout and
DMA'd out. Emissions are pre-gathered on the host (TRN2 has no fast dynamic
gather) and streamed in the exact consumption layout.
"""

import sys
import os

sys.path.insert(0, "/opt/trn_rl_repo")

import numpy as np

# ---- hardcoded geometry (from the problem spec) ----
Y = 64
XV = 50000
T = 1_000_000
NCORES = 8
TCORE = T // NCORES  # 125000

F = 160                 # chain-pairs per core (= chains per partition-group)
B = 2 * F               # 320 chains per core
L = 391                 # steps per chain; B*L = 125120 >= TCORE
WINDOWS = [128, 128, 128, 7]
D = 8                   # steps per emission DMA batch
NPX = 16                # chain-pairs per output staging DMA
BL = B * L              # padded output rows per core
WARM = 32               # host warmup steps

assert sum(WINDOWS) == L and B * L >= TCORE

LAST_RESULTS = None  # stashed BassKernelResults for test harness introspection

_CACHED_NC = None


def _build_bass():
    import concourse.tile as tile
    from concourse import bacc, mybir
    from contextlib import ExitStack

    f32 = mybir.dt.float32
    nc = bacc.Bacc("TRN2", target_bir_lowering=False)

    E = nc.dram_tensor("E", [128, L, F], f32, kind="ExternalInput")
    # CONST = [AB (128) | identity (128) | V (F)] packed so the kernel head
    # issues a single DMA wait (LDWEIGHTS tolerates only one sync wait).
    CONST = nc.dram_tensor("CONST", [128, 256 + F], f32, kind="ExternalInput")
    OUT = nc.dram_tensor("OUT", [BL, 64], f32, kind="ExternalOutput")

    with tile.TileContext(nc) as tc, ExitStack() as ctx:
        singles = ctx.enter_context(tc.tile_pool(name="singles", bufs=1))
        hist_p = ctx.enter_context(tc.tile_pool(name="hist", bufs=2))
        e_p = ctx.enter_context(tc.tile_pool(name="ebuf", bufs=2))
        stag_p = ctx.enter_context(tc.tile_pool(name="stag", bufs=2))
        ps_rec = ctx.enter_context(tc.tile_pool(name="psrec", bufs=4, space="PSUM"))
        ps_tp = ctx.enter_context(tc.tile_pool(name="pstp", bufs=2, space="PSUM"))

        const_sb = singles.tile([128, 256 + F], f32)
        nc.sync.dma_start(const_sb[:], CONST[:])
        ab_sb = const_sb[:, 0:128]
        id_sb = const_sb[:, 128:256]
        v_sb = const_sb[:, 256 : 256 + F]

        # OUT rows: t = (g*F + f)*L + (w0 + s)  ->  view (l, f, g, j)
        out_r = OUT[:].rearrange("(g f l) j -> l f g j", g=2, f=F)

        s_prev = v_sb
        w0 = 0
        for kw in WINDOWS:
            hist = hist_p.tile([128, F, 128], f32, tag="hist")
            for d0 in range(0, kw, D):
                dd = min(D, kw - d0)
                e_buf = e_p.tile([128, D, F], f32, tag="ebuf")
                nc.sync.dma_start(e_buf[:, :dd, :], E[:, w0 + d0 : w0 + d0 + dd, :])
                for s in range(d0, d0 + dd):
                    ps = ps_rec.tile([128, F], f32, tag="ps")
                    nc.tensor.matmul(ps[:], ab_sb, s_prev)
                    nc.vector.tensor_mul(
                        out=hist[:, :, s], in0=ps[:], in1=e_buf[:, s - d0, :]
                    )
                    s_prev = hist[:, :, s]
            # output stage for this window (overlaps next window's recurrence)
            for f0 in range(0, F, NPX):
                stag = stag_p.tile([128, NPX, 128], f32, tag="stag")
                for j4 in range(0, NPX, 4):
                    pt = ps_tp.tile([128, 4, 128], f32, tag="pt")
                    for j in range(4):
                        f = f0 + j4 + j
                        nc.tensor.transpose(pt[:kw, j, :], hist[:, f, :kw], id_sb)
                    nc.scalar.mul(
                        out=stag[:kw, j4 : j4 + 4, :], in_=pt[:kw, :, :], mul=1.0
                    )
                for g in range(2):
                    nc.sync.dma_start(
                        out_r[w0 : w0 + kw, f0 : f0 + NPX, g, :],
                        stag[:kw, :, g * 64 : (g + 1) * 64],
                    )
            w0 += kw
    nc.compile()
    return nc


def _prepare_inputs(x, transition, b, pi):
    """Host-side planning: emission pre-gather, chain seeds, constants."""
    A64 = transition.astype(np.float64)
    b64 = b.astype(np.float64)
    bs32 = (b * np.float32(XV)).astype(np.float32)  # prescaled emissions

    # pad x so padded chain tails index valid emissions
    pad = NCORES * 0 + (7 * TCORE + BL) - T  # = BL - TCORE
    x_pad = np.concatenate([x, np.repeat(x[-1:], pad)]).astype(np.int64)

    # ---- chain seeds: v_c ~ alpha_{start-1}; device step yields alpha_start ----
    starts = np.empty((NCORES, B), np.int64)
    for k in range(NCORES):
        starts[k] = k * TCORE + np.arange(B) * L
    flat_starts = starts.ravel()

    Vv = np.ones((NCORES * B, Y), np.float64) / Y
    warm_mask = flat_starts > 0
    widx = np.empty((warm_mask.sum(), WARM), np.int64)
    widx[:] = flat_starts[warm_mask, None] - WARM + np.arange(WARM)[None, :]
    bT64 = np.ascontiguousarray(b64.T)  # (XV, Y)
    EW = bT64[x_pad[widx]]  # (M, WARM, Y)
    Vw = Vv[warm_mask]
    for s in range(WARM):
        Vw = (Vw @ A64) * EW[:, s, :]
        Vw /= Vw.sum(1, keepdims=True)
    Vv[warm_mask] = Vw
    # global chain 0: A^T v = pi  so that (v @ A) * e0 == pi * e0 exactly
    Vv[0] = np.linalg.solve(A64.T, pi.astype(np.float64))
    Vv = Vv.astype(np.float32).reshape(NCORES, B, Y)

    # ---- constants: CONST = [AB | identity | V_core] ----
    ABm = np.zeros((128, 128), np.float32)
    ABm[:64, :64] = transition.astype(np.float32)
    ABm[64:, 64:] = transition.astype(np.float32)
    Im = np.eye(128, dtype=np.float32)

    # ---- per-core emission streams E[p, s, f] = bs[p%64, x[k*TCORE + c*L + s]],
    #      c = (p//64)*F + f ----
    in_maps = []
    for k in range(NCORES):
        idx = np.empty((B, L), np.int64)
        idx[:] = (k * TCORE + np.arange(B) * L)[:, None] + np.arange(L)[None, :]
        tok = x_pad[idx]  # (B, L) token ids
        Ek = np.empty((128, L, F), np.float32)
        # group g tokens arranged (L, F) then flat so np.take writes (64, L*F)
        for g in range(2):
            tg = np.ascontiguousarray(tok[g * F : (g + 1) * F].T)  # (L, F)
            np.take(bs32, tg.ravel(), axis=1, out=Ek[g * 64 : (g + 1) * 64].reshape(64, L * F))
        Ck = np.empty((128, 256 + F), np.float32)
        Ck[:, 0:128] = ABm
        Ck[:, 128:256] = Im
        Ck[:64, 256:] = Vv[k, :F].T
        Ck[64:, 256:] = Vv[k, F:].T
        in_maps.append({"E": Ek, "CONST": Ck})
    return in_maps


def kernel(x, transition, b, pi):
    global LAST_RESULTS, _CACHED_NC
    from concourse.bass_utils import run_bass_kernel_spmd

    in_maps = _prepare_inputs(
        np.asarray(x), np.asarray(transition), np.asarray(b), np.asarray(pi)
    )
    if _CACHED_NC is None:
        _CACHED_NC = _build_bass()
    res = run_bass_kernel_spmd(_CACHED_NC, in_maps, core_ids=list(range(NCORES)))
    LAST_RESULTS = res

    full = np.concatenate([r["OUT"][:TCORE] for r in res.results], axis=0)
    full = full / full.sum(axis=1, keepdims=True)
    return full.astype(np.float32)


# revision 8
# speedup vs baseline: 1.3391x; 1.3391x over previous
"""HMM scaled-forward (alpha scaling) kernel for Trainium2, 8 NeuronCores.

Math: alpha_t = normalize((alpha_{t-1} @ A) * b[:, x_t]).
The map v -> normalize((v @ A) * e) is a Hilbert-metric contraction (A is a
dense positive stochastic matrix; diagonal emission scaling is an isometry),
so the T=1M sequential scan is split into independent chains, each seeded by
a 32-step host-side warmup (converges to fp32 machine precision in ~16
steps). Per-step normalization is dropped on device (prescaled emissions
keep the unnormalized state within e^{+-10} over a chain); rows are
normalized on the host at the end.

Layout per core: GRP independent sub-batches (to pipeline PE<->DVE since
each sub-batch's recurrence is serial), each sub-batch packs 2x F chains
into 128 partitions (two 64-state groups, block-diag A).
Device per step and sub-batch: S = (S^T @ blockdiag(A,A)) * E (PE + DVE).
History is transposed per chain-pair on the PE into output-row layout,
copied PSUM->SBUF on ACT, and DMA'd out. Emissions are pre-gathered on the
host (TRN2 has no fast dynamic gather) and streamed in consumption order.
"""

import sys
import os

sys.path.insert(0, "/opt/trn_rl_repo")

import numpy as np

# ---- hardcoded geometry (from the problem spec) ----
Y = 64
XV = 50000
T = 1_000_000
NCORES = 8
TCORE = T // NCORES  # 125000

GRP = 2                 # independent sub-batches (PE<->DVE pipelining)
F = 112                 # chain-pairs per sub-batch
B = GRP * 2 * F         # 448 chains per core
L = 280                 # steps per chain; B*L = 125440 >= TCORE
WINDOWS = [96, 96, 88]
D = 8                   # steps per emission DMA batch
NPX = 16                # chain-pairs per output staging tile
BL = B * L              # padded output rows per core
WARM = 32               # host warmup steps

assert sum(WINDOWS) == L and B * L >= TCORE

LAST_RESULTS = None  # stashed BassKernelResults for test harness introspection

_CACHED_NC = None


def _build_bass():
    import concourse.tile as tile
    from concourse import bacc, mybir
    from contextlib import ExitStack

    f32 = mybir.dt.float32
    nc = bacc.Bacc("TRN2", target_bir_lowering=False)

    E = nc.dram_tensor("E", [GRP, 128, L, F], f32, kind="ExternalInput")
    # CONST = [AB (128) | identity (128) | V (GRP*F)] packed so the kernel
    # head issues a single DMA wait (LDWEIGHTS tolerates only one sync wait).
    CONST = nc.dram_tensor("CONST", [128, 256 + GRP * F], f32, kind="ExternalInput")
    OUT = nc.dram_tensor("OUT", [BL, 64], f32, kind="ExternalOutput")

    with tile.TileContext(nc) as tc, ExitStack() as ctx:
        singles = ctx.enter_context(tc.tile_pool(name="singles", bufs=1))
        hist_p = ctx.enter_context(tc.tile_pool(name="hist", bufs=2))
        e_p = ctx.enter_context(tc.tile_pool(name="ebuf", bufs=2))
        stag_p = ctx.enter_context(tc.tile_pool(name="stag", bufs=2))
        ps_rec = ctx.enter_context(tc.tile_pool(name="psrec", bufs=4, space="PSUM"))
        ps_tp = ctx.enter_context(tc.tile_pool(name="pstp", bufs=2, space="PSUM"))

        const_sb = singles.tile([128, 256 + GRP * F], f32)
        nc.sync.dma_start(const_sb[:], CONST[:])
        ab_sb = const_sb[:, 0:128]
        id_sb = const_sb[:, 128:256]

        # chain flat index c = (grp*2 + g)*F + f covers rows [c*L, (c+1)*L)
        out_r = OUT[:].rearrange("(grp g f l) j -> l grp g f j", grp=GRP, g=2, f=F)

        s_prev = [
            const_sb[:, 256 + grp * F : 256 + (grp + 1) * F] for grp in range(GRP)
        ]
        w0 = 0
        for kw in WINDOWS:
            hist = hist_p.tile([128, GRP, F, max(WINDOWS)], f32, tag="hist")
            for d0 in range(0, kw, D):
                dd = min(D, kw - d0)
                e_bufs = []
                for grp in range(GRP):
                    eb = e_p.tile([128, D, F], f32, tag=f"ebuf{grp}")
                    nc.sync.dma_start(
                        eb[:, :dd, :], E[grp, :, w0 + d0 : w0 + d0 + dd, :]
                    )
                    e_bufs.append(eb)
                for s in range(d0, d0 + dd):
                    for grp in range(GRP):
                        ps = ps_rec.tile([128, F], f32, tag="ps")
                        nc.tensor.matmul(ps[:], ab_sb, s_prev[grp])
                        nc.vector.tensor_mul(
                            out=hist[:, grp, :, s],
                            in0=ps[:],
                            in1=e_bufs[grp][:, s - d0, :],
                        )
                        s_prev[grp] = hist[:, grp, :, s]
            # output stage for this window (overlaps next window's recurrence)
            for grp in range(GRP):
                for f0 in range(0, F, NPX):
                    npx = min(NPX, F - f0)
                    stag = stag_p.tile([128, NPX, 128], f32, tag="stag")
                    for j4 in range(0, npx, 4):
                        n4 = min(4, npx - j4)
                        pt = ps_tp.tile([128, 4, 128], f32, tag="pt")
                        for j in range(n4):
                            f = f0 + j4 + j
                            nc.tensor.transpose(
                                pt[:kw, j, :], hist[:, grp, f, :kw], id_sb
                            )
                        nc.scalar.mul(
                            out=stag[:kw, j4 : j4 + n4, :],
                            in_=pt[:kw, :n4, :],
                            mul=1.0,
                        )
                    for g in range(2):
                        nc.sync.dma_start(
                            out_r[w0 : w0 + kw, grp, g, f0 : f0 + npx, :],
                            stag[:kw, :npx, g * 64 : (g + 1) * 64],
                        )
            w0 += kw
    nc.compile()
    return nc


def _prepare_inputs(x, transition, b, pi):
    """Host-side planning: emission pre-gather, chain seeds, constants."""
    A64 = transition.astype(np.float64)
    bs32 = (b * np.float32(XV)).astype(np.float32)  # prescaled emissions

    # pad x so padded chain tails index valid emissions
    pad = ((NCORES - 1) * TCORE + BL) - T  # = BL - TCORE
    x_pad = np.concatenate([x, np.repeat(x[-1:], pad)]).astype(np.int64)

    # ---- chain seeds: v_c ~ alpha_{start-1}; device step yields alpha_start ----
    starts = np.empty((NCORES, B), np.int64)
    for k in range(NCORES):
        starts[k] = k * TCORE + np.arange(B) * L
    flat_starts = starts.ravel()

    Vv = np.ones((NCORES * B, Y), np.float64) / Y
    warm_mask = flat_starts > 0
    widx = np.empty((warm_mask.sum(), WARM), np.int64)
    widx[:] = flat_starts[warm_mask, None] - WARM + np.arange(WARM)[None, :]
    bT64 = np.ascontiguousarray(b.astype(np.float64).T)  # (XV, Y)
    EW = bT64[x_pad[widx]]  # (M, WARM, Y)
    Vw = Vv[warm_mask]
    for s in range(WARM):
        Vw = (Vw @ A64) * EW[:, s, :]
        Vw /= Vw.sum(1, keepdims=True)
    Vv[warm_mask] = Vw
    # global chain 0: A^T v = pi  so that (v @ A) * e0 == pi * e0 exactly
    Vv[0] = np.linalg.solve(A64.T, pi.astype(np.float64))
    Vv = Vv.astype(np.float32).reshape(NCORES, B, Y)

    ABm = np.zeros((128, 128), np.float32)
    ABm[:64, :64] = transition.astype(np.float32)
    ABm[64:, 64:] = transition.astype(np.float32)
    Im = np.eye(128, dtype=np.float32)

    # ---- per-core emission streams:
    # E[grp, g*64+j, s, f] = bs[j, x[k*TCORE + c*L + s]],  c = (grp*2+g)*F + f
    in_maps = []
    for k in range(NCORES):
        idx = np.empty((B, L), np.int64)
        idx[:] = (k * TCORE + np.arange(B) * L)[:, None] + np.arange(L)[None, :]
        tok = x_pad[idx]  # (B, L) token ids
        Ek = np.empty((GRP, 128, L, F), np.float32)
        for grp in range(GRP):
            for g in range(2):
                c0 = (grp * 2 + g) * F
                tg = np.ascontiguousarray(tok[c0 : c0 + F].T)  # (L, F)
                np.take(
                    bs32,
                    tg.ravel(),
                    axis=1,
                    out=Ek[grp, g * 64 : (g + 1) * 64].reshape(64, L * F),
                )
        Ck = np.empty((128, 256 + GRP * F), np.float32)
        Ck[:, 0:128] = ABm
        Ck[:, 128:256] = Im
        for grp in range(GRP):
            for g in range(2):
                c0 = (grp * 2 + g) * F
                Ck[g * 64 : (g + 1) * 64, 256 + grp * F : 256 + (grp + 1) * F] = Vv[
                    k, c0 : c0 + F
                ].T
        in_maps.append({"E": Ek, "CONST": Ck})
    return in_maps


def kernel(x, transition, b, pi):
    global LAST_RESULTS, _CACHED_NC
    from concourse.bass_utils import run_bass_kernel_spmd

    in_maps = _prepare_inputs(
        np.asarray(x), np.asarray(transition), np.asarray(b), np.asarray(pi)
    )
    if _CACHED_NC is None:
        _CACHED_NC = _build_bass()
    res = run_bass_kernel_spmd(_CACHED_NC, in_maps, core_ids=list(range(NCORES)))
    LAST_RESULTS = res

    full = np.concatenate([r["OUT"][:TCORE] for r in res.results], axis=0)
    full = full / full.sum(axis=1, keepdims=True)
    return full.astype(np.float32)
